# revision 4
# baseline (speedup 1.0000x reference)
"""DMPNN layer kernel for 8 Trainium2 NeuronCores.

Sharding: data-parallel over destination nodes j (dim 2 of edge_attr/adj,
dim 1 of the output). Each core gets a 64-column j-slice of edge_attr/adj,
the full h (needed because messages sum over all source nodes i), and the
small weights replicated. The batch-global mask (adj.sum(0) > 0) only needs
the core's own j-slice of adj over the full batch, so no collective at all.

Math per core (j in its 64-column slice, source nodes i = 4p + q):
  mask[i,j]   = max_b adj[b,i,j]                    (adj is 0/1)
  deg[j]      = sum_i mask[i,j]
  mh[b,j,f]   = sum_i mask[i,j] h[b,i,f]
  me[b,j,e]   = sum_i mask[i,j] edge[b,i,j,e]
  msg[b,j,o]  = sum_f Wh[o,f] mh[b,j,f] + deg[j] wb[o] + sum_e We[o,e] me[b,j,e]
  out[b,j,o]  = sum_f U[o,f] (h[b,j,f] + msg[b,j,f]) + ub[o]

v2 changes vs v1 (82 us):
 - All heavy TensorE traffic in bf16 (fp32 matmul runs LOW_HIGH = 2 passes);
   the DVE mask-multiply casts fp32->bf16 on the write for free.
 - The whole working set is preloaded into SBUF with large coalesced DMAs
   (edge: 8x1MiB on the sync HWDGE queue; adj: 1 DMA on the scalar queue;
   h: one casting SWDGE DMA on gpsimd), so the DMA stream runs at line rate
   instead of trickling behind the PE.
 - The (j,e)->[e,j] DRAM bounce for the reduced edge messages is gone:
   msg_edge is accumulated as 8 rank-1 matmuls lhsT=We[:,e] (1 row of WeM),
   rhs=me_sb[0, e::8] straight into the message PSUM tile.
"""

import numpy as np


def _ensure_path():
    try:
        import concourse.bass  # noqa: F401
    except ImportError:
        import sys

        for p in ("/opt/trn_rl_repo", "/root/.axon_site/_ro/trn_rl_repo"):
            if p not in sys.path:
                sys.path.insert(0, p)


B, N, H, E = 8, 512, 64, 8
NCORES = 8
JB = N // NCORES  # 64 destination columns per core
CH = N // 128  # 4 source-node sub-chunks (i = 4p + q)


_CACHE = {}


def _build_program():
    _ensure_path()
    import concourse.bacc as bacc
    import concourse.mybir as mybir
    import concourse.tile as tile

    dt = mybir.dt
    f32 = dt.float32
    bf16 = dt.bfloat16
    i32 = dt.int32
    Alu = mybir.AluOpType

    nc = bacc.Bacc("TRN2", debug=False, num_devices=NCORES)

    edge = nc.dram_tensor("edge", [B, N, JB, E], f32, kind="ExternalInput").ap()
    adjs = nc.dram_tensor("adjs", [B, N, JB], i32, kind="ExternalInput").ap()
    h = nc.dram_tensor("h", [B, N, H], f32, kind="ExternalInput").ap()
    hs = nc.dram_tensor("hs", [B, JB, H], f32, kind="ExternalInput").ap()
    Ww = nc.dram_tensor("Ww", [H, H + E], f32, kind="ExternalInput").ap()
    Wb = nc.dram_tensor("Wb", [1, H], f32, kind="ExternalInput").ap()
    Uw = nc.dram_tensor("Uw", [H, H], f32, kind="ExternalInput").ap()
    Ub = nc.dram_tensor("Ub", [1, H], f32, kind="ExternalInput").ap()
    out = nc.dram_tensor("out", [B, H, JB], f32, kind="ExternalOutput").ap()

    ident_d = nc.inline_tensor(np.eye(128, dtype=np.float32), "ident")

    with tile.TileContext(nc) as tc:
        with (
            tc.tile_pool(name="const", bufs=1) as cpool,
            tc.tile_pool(name="masked", bufs=3) as mpool,
            tc.tile_pool(name="small", bufs=3) as spool,
            tc.tile_pool(name="pe", bufs=2, space="PSUM") as ppool_e,
            tc.tile_pool(name="pmh", bufs=2, space="PSUM") as ppool_mh,
            tc.tile_pool(name="pmsg", bufs=2, space="PSUM") as ppool_msg,
            tc.tile_pool(name="pout", bufs=2, space="PSUM") as ppool_out,
        ):
            # ---- adj first on the scalar HWDGE queue: it gates the mask ----
            adj_sb = cpool.tile([128, B * CH * JB], i32)
            nc.scalar.dma_start(
                out=adj_sb.rearrange("p (b qj) -> p b qj", b=B),
                in_=adjs.rearrange("b (p q) j -> p b (q j)", q=CH),
            )

            # ---- edge: the 8 MiB stream, alone on the sync HWDGE queue ----
            # each per-batch slab is one fully contiguous 8 KiB run per
            # partition (i = 4p + q, q-major), so descriptors coalesce.
            edge_t = [cpool.tile([128, CH * JB * E], f32, name=f"edge{b}") for b in range(B)]
            for b in range(B):
                nc.sync.dma_start(
                    out=edge_t[b][:, :],
                    in_=edge[b].rearrange("(p q) j e -> p (q j e)", q=CH),
                )

            # ---- h with fp32->bf16 cast on the gpsimd SWDGE queue ----
            h_bf = cpool.tile([128, B * CH * H], bf16)
            nc.gpsimd.dma_start(
                out=h_bf.rearrange("p (b qf) -> p b qf", b=B),
                in_=h.rearrange("b (p q) f -> p b (q f)", q=CH),
            )

            # ---- small loads on the scalar queue ----
            hs_all = cpool.tile([JB, B * H], f32)
            nc.scalar.dma_start(
                out=hs_all.rearrange("j (b f) -> j b f", b=B),
                in_=hs.rearrange("b j f -> j b f"),
            )
            ident = cpool.tile([128, 128], f32)
            nc.scalar.dma_start(out=ident[:, :], in_=ident_d.ap()[:, :])
            Ww_sb = cpool.tile([H, H + E], f32)
            nc.scalar.dma_start(out=Ww_sb[:, :], in_=Ww[:, :])
            Uw_sb = cpool.tile([H, H], f32)
            nc.scalar.dma_start(out=Uw_sb[:, :], in_=Uw[:, :])
            wb_sb = cpool.tile([1, H], f32)
            nc.scalar.dma_start(out=wb_sb[:, :], in_=Wb[:, :])
            ub_sb = cpool.tile([1, H], f32)
            nc.scalar.dma_start(out=ub_sb[:, :], in_=Ub[:, :])

            # ---- constants ----
            ones_stat = cpool.tile([128, 1], f32)
            nc.vector.memset(ones_stat[:, :], 1.0)
            ones_bf = cpool.tile([128, 1], bf16)
            nc.vector.memset(ones_bf[:, :], 1.0)
            ones_row = cpool.tile([1, JB], f32)
            nc.vector.memset(ones_row[:, :], 1.0)

            # ---- mask: pairwise max tree over the batch axis (adj is 0/1) --
            adj_v = adj_sb.rearrange("p (b qj) -> p b qj", b=B)
            mt0 = cpool.tile([128, CH * JB], i32, name="mt0")
            mt1 = cpool.tile([128, CH * JB], i32, name="mt1")
            mt2 = cpool.tile([128, CH * JB], i32, name="mt2")
            mt3 = cpool.tile([128, CH * JB], i32, name="mt3")
            nc.vector.tensor_tensor(mt0[:, :], adj_v[:, 0], adj_v[:, 1], Alu.max)
            nc.vector.tensor_tensor(mt1[:, :], adj_v[:, 2], adj_v[:, 3], Alu.max)
            nc.vector.tensor_tensor(mt2[:, :], adj_v[:, 4], adj_v[:, 5], Alu.max)
            nc.vector.tensor_tensor(mt3[:, :], adj_v[:, 6], adj_v[:, 7], Alu.max)
            nc.vector.tensor_tensor(mt0[:, :], mt0[:, :], mt1[:, :], Alu.max)
            nc.vector.tensor_tensor(mt2[:, :], mt2[:, :], mt3[:, :], Alu.max)
            mask = cpool.tile([128, CH * JB], f32)
            nc.vector.tensor_tensor(mask[:, :], mt0[:, :], mt2[:, :], Alu.max)
            mask_bf = cpool.tile([128, CH * JB], bf16)
            nc.vector.tensor_copy(mask_bf[:, :], mask[:, :])

            # ---- weight transposes: Wh2 = Wh^T, U2 = U^T ------
            Wh2 = cpool.tile([H, H], bf16)
            U2 = cpool.tile([H, H], bf16)

            pwh = ppool_out.tile([H, H], f32, tag="o", name="pwh")
            nc.tensor.transpose(pwh[:, :], Ww_sb[:, 0:H], ident[0:H, 0:H])
            nc.vector.tensor_copy(Wh2[:, :], pwh[:, :])

            puw = ppool_out.tile([H, H], f32, tag="o", name="puw")
            nc.tensor.transpose(puw[:, :], Uw_sb[:, :], ident[0:H, 0:H])
            nc.vector.tensor_copy(U2[:, :], puw[:, :])

            # We rows flattened on partition 0: WeM_flat[0, e*H + o] = We[o, e]
            # (so each rank-1 lhsT slice in s3 has base_partition 0)
            WeM_f32 = cpool.tile([1, E * H], f32)
            nc.scalar.dma_start(
                out=WeM_f32.rearrange("p (e o) -> p e o", e=E),
                in_=Ww[:, H : H + E].rearrange("o e -> e o"),
            )
            WeM_flat = cpool.tile([1, E * H], bf16)
            nc.vector.tensor_copy(WeM_flat[:, :], WeM_f32[:, :])

            # hs transposed: hsT_all[f, (b j)]
            hsT_all = cpool.tile([H, B * JB], f32)
            for b in range(B):
                pht = ppool_msg.tile([H, JB], f32, tag="m", name="pht")
                nc.tensor.transpose(
                    pht[:, :], hs_all[:, b * H : (b + 1) * H], ident[0:JB, 0:JB]
                )
                nc.vector.tensor_copy(hsT_all[:, b * JB : (b + 1) * JB], pht[:, :])

            # deg as a row vector [1, j] = ones^T @ mask
            psum_deg = ppool_out.tile([1, JB], f32, tag="o", name="psum_deg")
            for c in range(CH):
                nc.tensor.matmul(
                    psum_deg[:, :],
                    lhsT=ones_stat[:, :],
                    rhs=mask[:, c * JB : (c + 1) * JB],
                    start=(c == 0),
                    stop=(c == CH - 1),
                )
            deg_sb = cpool.tile([1, JB], f32)
            nc.scalar.copy(deg_sb[:, :], psum_deg[:, :])

            # broadcast view of the mask over the e axis (stride-0)
            mask_bcast = mask.rearrange("p (q j) -> p q j", q=CH).broadcast_to(
                [128, CH, JB, E]
            )

            # ---------------- per-batch software pipeline ----------------
            st = [dict() for _ in range(B)]

            def s1(b):
                # mh matmuls first: they only need mask_bf + h_bf
                psum_mhT = ppool_mh.tile([H, JB], f32, name="psum_mhT")
                for c in range(CH):
                    nc.tensor.matmul(
                        psum_mhT[:, :],
                        lhsT=h_bf[:, (b * CH + c) * H : (b * CH + c + 1) * H],
                        rhs=mask_bf[:, c * JB : (c + 1) * JB],
                        start=(c == 0),
                        stop=(c == CH - 1),
                    )
                st[b]["psum_mhT"] = psum_mhT

                # heavy streaming: mask multiply (cast to bf16) + i-reduction
                masked = mpool.tile([128, CH * JB * E], bf16, name="masked")
                mk_v = masked.rearrange("p (q j e) -> p q j e", q=CH, j=JB)
                eg_v = edge_t[b].rearrange("p (q j e) -> p q j e", q=CH, j=JB)
                psum_e = ppool_e.tile([1, JB * E], f32, name="psum_e")
                for c in range(CH):
                    nc.vector.tensor_tensor(
                        out=mk_v[:, c],
                        in0=eg_v[:, c],
                        in1=mask_bcast[:, c],
                        op=Alu.mult,
                    )
                    nc.tensor.matmul(
                        psum_e[:, :],
                        lhsT=ones_bf[:, :],
                        rhs=masked[:, c * JB * E : (c + 1) * JB * E],
                        start=(c == 0),
                        stop=(c == CH - 1),
                    )
                st[b]["psum_e"] = psum_e

            def s2(b):
                # PSUM extraction (ACT engine), casting to bf16 for s3
                d = st[b]
                me_sb = spool.tile([1, JB * E], bf16, name="me_sb")
                nc.scalar.copy(me_sb[:, :], d["psum_e"][:, :])
                d["me_sb"] = me_sb
                mhT_s = spool.tile([H, JB], bf16, name="mhT_s")
                nc.scalar.copy(mhT_s[:, :], d["psum_mhT"][:, :])
                d["mhT_s"] = mhT_s

            def s3(b):
                # messages + update + output
                d = st[b]
                me_v = d["me_sb"].rearrange("p (j e) -> p e j", e=E)
                psum_msgT = ppool_msg.tile([H, JB], f32, tag="m", name="psum_msgT")
                nc.tensor.matmul(
                    psum_msgT[:, :], lhsT=Wh2[:, :], rhs=d["mhT_s"][:, :],
                    start=True, stop=False,
                )
                # msg_edge: 8 rank-1 updates  We[:,e] x me[., e]
                for e in range(E):
                    nc.tensor.matmul(
                        psum_msgT[:, :],
                        lhsT=WeM_flat[:, e * H : (e + 1) * H],
                        rhs=me_v[:, e],
                        start=False, stop=False,
                    )
                nc.tensor.matmul(
                    psum_msgT[:, :], lhsT=wb_sb[:, :], rhs=deg_sb[:, :],
                    start=False, stop=True,
                )
                XT_s = spool.tile([H, JB], bf16, name="XT_s")
                nc.vector.tensor_tensor(
                    out=XT_s[:, :],
                    in0=psum_msgT[:, :],
                    in1=hsT_all[:, b * JB : (b + 1) * JB],
                    op=Alu.add,
                )
                psum_outT = ppool_out.tile([H, JB], f32, tag="o", name="psum_outT")
                nc.tensor.matmul(
                    psum_outT[:, :], lhsT=U2[:, :], rhs=XT_s[:, :],
                    start=True, stop=False,
                )
                nc.tensor.matmul(
                    psum_outT[:, :], lhsT=ub_sb[:, :], rhs=ones_row[:, :],
                    start=False, stop=True,
                )
                out_sb = spool.tile([H, JB], f32, name="out_sb")
                nc.scalar.copy(out_sb[:, :], psum_outT[:, :])
                nc.scalar.dma_start(out=out[b], in_=out_sb[:, :])

            LAG2, LAG3 = 1, 2
            for b in range(B + LAG3):
                if b < B:
                    s1(b)
                if LAG2 <= b < B + LAG2:
                    s2(b - LAG2)
                if LAG3 <= b:
                    s3(b - LAG3)

    nc.compile()
    return nc


def _get_program():
    if "nc" not in _CACHE:
        _CACHE["nc"] = _build_program()
    return _CACHE["nc"]


def _make_in_maps(h, edge_attr, adj, W_w, W_b, U_w, U_b):
    h = np.ascontiguousarray(np.asarray(h, dtype=np.float32))
    edge_attr = np.asarray(edge_attr, dtype=np.float32)
    adj = np.asarray(adj, dtype=np.int32)
    W_w = np.ascontiguousarray(np.asarray(W_w, dtype=np.float32))
    W_b = np.ascontiguousarray(np.asarray(W_b, dtype=np.float32)).reshape(1, H)
    U_w = np.ascontiguousarray(np.asarray(U_w, dtype=np.float32))
    U_b = np.ascontiguousarray(np.asarray(U_b, dtype=np.float32)).reshape(1, H)

    in_maps = []
    for c in range(NCORES):
        j0 = c * JB
        in_maps.append(
            {
                "edge": np.ascontiguousarray(edge_attr[:, :, j0 : j0 + JB, :]),
                "adjs": np.ascontiguousarray(adj[:, :, j0 : j0 + JB]),
                "h": h,
                "hs": np.ascontiguousarray(h[:, j0 : j0 + JB, :]),
                "Ww": W_w,
                "Wb": W_b,
                "Uw": U_w,
                "Ub": U_b,
            }
        )
    return in_maps


def _install_ntff_hook():
    """The agent image lacks antenv.axon_hooks; synthesize it so trace=True
    can reach the libaxon NTFF profiling entry points."""
    import sys
    import types

    try:
        from antenv.axon_hooks import get_axon_ntff_profile_hook  # noqa: F401

        return
    except ImportError:
        pass
    import antenv

    mod = types.ModuleType("antenv.axon_hooks")
    _h = [None]
    mod.set_axon_ntff_profile_hook = lambda hook: _h.__setitem__(0, hook)
    mod.get_axon_ntff_profile_hook = lambda: _h[0]
    sys.modules["antenv.axon_hooks"] = mod
    antenv.axon_hooks = mod
    try:
        from trn_agent_boot.trn_boot import _ntff_profile_via_ctypes

        mod.set_axon_ntff_profile_hook(
            _ntff_profile_via_ctypes("/opt/axon/libaxon_pjrt.so")
        )
    except Exception:
        pass
    # avoid the bucket upload (no bucket in this container)
    import concourse.bass_utils as bu

    bu.upload_artifacts = lambda tmpdir: str(tmpdir)


def run(h, edge_attr, adj, W_w, W_b, U_w, U_b, trace=False, trace_cores=None):
    """Run the kernel; returns (output, BassKernelResults)."""
    _ensure_path()
    if trace:
        _install_ntff_hook()
    from concourse.bass_utils import run_bass_kernel_spmd

    nc = _get_program()
    in_maps = _make_in_maps(h, edge_attr, adj, W_w, W_b, U_w, U_b)
    kw = {}
    if trace:
        kw = {"trace": True, "trace_cores": trace_cores or [0]}
    res = run_bass_kernel_spmd(nc, in_maps, list(range(NCORES)), **kw)
    outs = [res.results[c]["out"].transpose(0, 2, 1) for c in range(NCORES)]
    full = np.concatenate(outs, axis=1)  # [B, N, H]
    return full, res


def kernel(h, edge_attr, adj, W_w, W_b, U_w, U_b):
    full, _ = run(h, edge_attr, adj, W_w, W_b, U_w, U_b)
    return full


# revision 6
# speedup vs baseline: 1.1570x; 1.1570x over previous
"""DMPNN layer kernel for 8 Trainium2 NeuronCores.

Sharding: data-parallel over destination nodes j (dim 2 of edge_attr/adj,
dim 1 of the output). Each core gets a 64-column j-slice of edge_attr/adj,
the full h (needed because messages sum over all source nodes i), and the
small weights replicated. The batch-global mask (adj.sum(0) > 0) only needs
the core's own j-slice of adj over the full batch, so no collective at all.

Math per core (j in its 64-column slice, source nodes i = 4p + q):
  mask[i,j]   = max_b adj[b,i,j]                    (adj is 0/1)
  deg[j]      = sum_i mask[i,j]
  mh[b,j,f]   = sum_i mask[i,j] h[b,i,f]
  me[b,j,e]   = sum_i mask[i,j] edge[b,i,j,e]
  msg[b,j,o]  = sum_f Wh[o,f] mh[b,j,f] + deg[j] wb[o] + sum_e We[o,e] me[b,j,e]
  out[b,j,o]  = sum_f U[o,f] (h[b,j,f] + msg[b,j,f]) + ub[o]

v3 design notes (PE ran cold at ~1.2 GHz with ~165+ ns/instr overhead, DMA
issue costs its engine ~0.5-1 us, so: few instructions, bf16 everywhere hot,
big DMAs, balanced engines):
 - adj first on the sync HWDGE queue, then the 8x1MiB edge stream; h/hs/
   weights on the scalar queue; per-batch tiny DMAs stay on scalar so they
   never queue behind the edge stream.
 - mask via one DVE tensor_reduce(max) over the batch axis.
 - DVE does the mask-multiply (fp32 in, bf16 out) in 2 ops/batch plus a
   2-way q-fold (bf16 adds), so the PE i-reduction is 2 matmuls of 512
   columns instead of 4, in bf16.
 - The entire message matmul is ONE PE instruction: stacked lhsT
   [Wh^T; wb; We^T] (73 rows) against rhs [mhT; deg; me^T] (73 rows).
   The rhs stack is assembled by tiny SBUF->SBUF DMAs (partition moves)
   plus ACT copies; me^T comes out of the PSUM copy pre-transposed by a
   strided write AP. The output matmul is [U^T; ub] (65 rows) against
   [XT; ones].
 - mh matmuls for ALL batches run in a prologue (they need only adj+h),
   filling the PE while the edge stream warms up.
"""

import numpy as np


def _ensure_path():
    try:
        import concourse.bass  # noqa: F401
    except ImportError:
        import sys

        for p in ("/opt/trn_rl_repo", "/root/.axon_site/_ro/trn_rl_repo"):
            if p not in sys.path:
                sys.path.insert(0, p)


B, N, H, E = 8, 512, 64, 8
NCORES = 8
JB = N // NCORES  # 64 destination columns per core
CH = N // 128  # 4 source-node sub-chunks (i = 4p + q)


_CACHE = {}


def _build_program():
    _ensure_path()
    import concourse.bacc as bacc
    import concourse.mybir as mybir
    import concourse.tile as tile

    dt = mybir.dt
    f32 = dt.float32
    bf16 = dt.bfloat16
    i32 = dt.int32
    Alu = mybir.AluOpType
    Axis = mybir.AxisListType

    nc = bacc.Bacc("TRN2", debug=False, num_devices=NCORES)

    edge = nc.dram_tensor("edge", [B, N, JB, E], f32, kind="ExternalInput").ap()
    adjs = nc.dram_tensor("adjs", [B, N, JB], i32, kind="ExternalInput").ap()
    h = nc.dram_tensor("h", [B, N, H], f32, kind="ExternalInput").ap()
    hs = nc.dram_tensor("hs", [B, JB, H], f32, kind="ExternalInput").ap()
    Ww = nc.dram_tensor("Ww", [H, H + E], f32, kind="ExternalInput").ap()
    Wb = nc.dram_tensor("Wb", [1, H], f32, kind="ExternalInput").ap()
    Uw = nc.dram_tensor("Uw", [H, H], f32, kind="ExternalInput").ap()
    Ub = nc.dram_tensor("Ub", [1, H], f32, kind="ExternalInput").ap()
    out = nc.dram_tensor("out", [B, H, JB], f32, kind="ExternalOutput").ap()

    ident_d = nc.inline_tensor(np.eye(128, dtype=np.float32), "ident")

    KM = H + 1 + E  # 73 contraction rows of the fused message matmul
    KU = H + 1  # 65 contraction rows of the fused output matmul

    with tile.TileContext(nc) as tc:
        with (
            tc.tile_pool(name="const", bufs=1) as cpool,
            tc.tile_pool(name="masked", bufs=3) as mpool,
            tc.tile_pool(name="acc", bufs=4) as apool,
            tc.tile_pool(name="small", bufs=3) as spool,
            tc.tile_pool(name="pe", bufs=2, space="PSUM") as ppool_e,
            tc.tile_pool(name="pmh", bufs=2, space="PSUM") as ppool_mh,
            tc.tile_pool(name="pmsg", bufs=2, space="PSUM") as ppool_msg,
            tc.tile_pool(name="pout", bufs=2, space="PSUM") as ppool_out,
        ):
            # ---- adj first on sync: it gates the mask -> everything ----
            adj_sb = cpool.tile([128, B * CH * JB], i32)
            nc.sync.dma_start(
                out=adj_sb.rearrange("p (b qj) -> p b qj", b=B),
                in_=adjs.rearrange("b (p q) j -> p b (q j)", q=CH),
            )

            # ---- edge: the 8 MiB stream on sync; contiguous 8 KiB/partition
            edge_t = [
                cpool.tile([128, CH * JB * E], f32, name=f"edge{b}") for b in range(B)
            ]
            for b in range(B):
                nc.sync.dma_start(
                    out=edge_t[b][:, :],
                    in_=edge[b].rearrange("(p q) j e -> p (q j e)", q=CH),
                )

            # ---- small loads + h on the scalar queue ----
            ident = cpool.tile([128, 128], f32)
            nc.scalar.dma_start(out=ident[:, :], in_=ident_d.ap()[:, :])
            Ww_sb = cpool.tile([H, H + E], f32)
            nc.scalar.dma_start(out=Ww_sb[:, :], in_=Ww[:, :])
            Uw_sb = cpool.tile([H, H], f32)
            nc.scalar.dma_start(out=Uw_sb[:, :], in_=Uw[:, :])
            wb_sb = cpool.tile([1, H], f32)
            nc.scalar.dma_start(out=wb_sb[:, :], in_=Wb[:, :])
            ub_sb = cpool.tile([1, H], f32)
            nc.scalar.dma_start(out=ub_sb[:, :], in_=Ub[:, :])
            hs_all = cpool.tile([JB, B * H], f32)
            nc.scalar.dma_start(
                out=hs_all.rearrange("j (b f) -> j b f", b=B),
                in_=hs.rearrange("b j f -> j b f"),
            )
            h_f32 = cpool.tile([128, B * CH * H], f32)
            nc.scalar.dma_start(
                out=h_f32.rearrange("p (b qf) -> p b qf", b=B),
                in_=h.rearrange("b (p q) f -> p b (q f)", q=CH),
            )

            # ---- constants ----
            ones_bf = cpool.tile([128, 1], bf16)
            nc.vector.memset(ones_bf[:, :], 1.0)

            # ---- mask: one segmented max-reduce over the batch axis ----
            mask_f = cpool.tile([128, CH * JB], f32)
            nc.vector.tensor_reduce(
                out=mask_f[:, :],
                in_=adj_sb.rearrange("p (b qj) -> p qj b", b=B),
                axis=Axis.X,
                op=Alu.max,
            )
            mask_bf = cpool.tile([128, CH * JB], bf16)
            nc.vector.tensor_copy(mask_bf[:, :], mask_f[:, :])

            # ---- h cast to bf16 on ACT ----
            h_bf = cpool.tile([128, B * CH * H], bf16)
            nc.scalar.copy(h_bf[:, :], h_f32[:, :])

            # ---- weight transposes + bf16 casts ----
            Wh2 = cpool.tile([H, H], bf16)
            WeM8 = cpool.tile([E, H], bf16)
            U2 = cpool.tile([H, H], bf16)
            wb_bf = cpool.tile([1, H], bf16)
            ub_bf = cpool.tile([1, H], bf16)
            nc.vector.tensor_copy(wb_bf[:, :], wb_sb[:, :])
            nc.vector.tensor_copy(ub_bf[:, :], ub_sb[:, :])

            pwh = ppool_out.tile([H, H], f32, tag="o", name="pwh")
            nc.tensor.transpose(pwh[:, :], Ww_sb[:, 0:H], ident[0:H, 0:H])
            nc.vector.tensor_copy(Wh2[:, :], pwh[:, :])

            pwe = ppool_out.tile([E, H], f32, tag="o", name="pwe")
            nc.tensor.transpose(pwe[:, :], Ww_sb[:, H : H + E], ident[0:H, 0:H])
            nc.vector.tensor_copy(WeM8[:, :], pwe[:, :])

            puw = ppool_out.tile([H, H], f32, tag="o", name="puw")
            nc.tensor.transpose(puw[:, :], Uw_sb[:, :], ident[0:H, 0:H])
            nc.vector.tensor_copy(U2[:, :], puw[:, :])

            # hs transposed: hsT_all[f, (b j)]
            hsT_all = cpool.tile([H, B * JB], f32)
            for b in range(B):
                pht = ppool_msg.tile([H, JB], f32, tag="m", name="pht")
                nc.tensor.transpose(
                    pht[:, :], hs_all[:, b * H : (b + 1) * H], ident[0:JB, 0:JB]
                )
                nc.vector.tensor_copy(hsT_all[:, b * JB : (b + 1) * JB], pht[:, :])

            # deg[j] = sum_i mask[i,j] (exact: 0/1 in bf16, f32 PSUM accum)
            psum_deg = ppool_out.tile([1, JB], f32, tag="o", name="psum_deg")
            for c in range(CH):
                nc.tensor.matmul(
                    psum_deg[:, :],
                    lhsT=ones_bf[:, :],
                    rhs=mask_bf[:, c * JB : (c + 1) * JB],
                    start=(c == 0),
                    stop=(c == CH - 1),
                )
            deg_bf = cpool.tile([1, JB], bf16)
            nc.scalar.copy(deg_bf[:, :], psum_deg[:, :])

            # ---- stacked stationary operands (partition moves via DMA) ----
            WWb = cpool.tile([KM, H], bf16)  # [Wh^T; wb; We^T]
            nc.scalar.dma_start(out=WWb[0:H, :], in_=Wh2[:, :])
            nc.scalar.dma_start(out=WWb[H : H + 1, :], in_=wb_bf[:, :])
            nc.scalar.dma_start(out=WWb[H + 1 : KM, :], in_=WeM8[:, :])
            UUb = cpool.tile([KU, H], bf16)  # [U^T; ub]
            nc.scalar.dma_start(out=UUb[0:H, :], in_=U2[:, :])
            nc.scalar.dma_start(out=UUb[H : H + 1, :], in_=ub_bf[:, :])

            # stacked rhs buffers: mhTd[b] = [mhT; deg; me^T]
            mhTd = [cpool.tile([KM, JB], bf16, name=f"mhTd{b}") for b in range(B)]
            for b in range(B):
                nc.scalar.dma_start(out=mhTd[b][H : H + 1, :], in_=deg_bf[:, :])

            # XT buffers: [msgT + hsT; ones]
            NXT = 3
            XT = [cpool.tile([KU, JB], bf16, name=f"XT{b}") for b in range(NXT)]
            for i in range(NXT):
                nc.vector.memset(XT[i][H : H + 1, :], 1.0)

            # ---- mh prologue: all batches (needs only adj + h) ----
            for b in range(B):
                psum_mhT = ppool_mh.tile([H, JB], f32, name="psum_mhT")
                for c in range(CH):
                    nc.tensor.matmul(
                        psum_mhT[:, :],
                        lhsT=h_bf[:, (b * CH + c) * H : (b * CH + c + 1) * H],
                        rhs=mask_bf[:, c * JB : (c + 1) * JB],
                        start=(c == 0),
                        stop=(c == CH - 1),
                    )
                nc.scalar.copy(mhTd[b][0:H, :], psum_mhT[:, :])

            # broadcast view of the f32 mask over the e axis (stride-0)
            mask_q = mask_f.rearrange("p (q j) -> p q j", q=CH)

            # ---------------- per-batch software pipeline ----------------
            st = [dict() for _ in range(B)]
            HALF = CH // 2 * JB * E  # 1024: two q-chunks per mult op

            def s1(b):
                masked = mpool.tile([128, CH * JB * E], bf16, name="masked")
                mk_v = masked.rearrange("p (q j e) -> p q j e", q=CH, j=JB)
                eg_v = edge_t[b].rearrange("p (q j e) -> p q j e", q=CH, j=JB)
                psum_e = ppool_e.tile([1, JB * E], f32, name="psum_e")
                d = st[b]
                for half in range(2):
                    q0 = 2 * half
                    nc.vector.tensor_tensor(
                        out=mk_v[:, q0 : q0 + 2],
                        in0=eg_v[:, q0 : q0 + 2],
                        in1=mask_q[:, q0 : q0 + 2].broadcast_to([128, 2, JB, E]),
                        op=Alu.mult,
                    )
                    acc = apool.tile([128, JB * E], bf16, name="acc")
                    nc.vector.tensor_tensor(
                        out=acc[:, :],
                        in0=masked[:, q0 * JB * E : (q0 + 1) * JB * E],
                        in1=masked[:, (q0 + 1) * JB * E : (q0 + 2) * JB * E],
                        op=Alu.add,
                    )
                    nc.tensor.matmul(
                        psum_e[:, :],
                        lhsT=ones_bf[:, :],
                        rhs=acc[:, :],
                        start=(half == 0),
                        stop=(half == 1),
                    )
                d["psum_e"] = psum_e

            def s2(b):
                # PSUM -> SBUF with the (j,e)->(e,j) remap folded into the
                # write AP, then an 8-descriptor partition-move DMA into the
                # stacked rhs.
                d = st[b]
                me_sb = spool.tile([1, JB * E], bf16, name="me_sb")
                nc.scalar.copy(
                    out=me_sb.rearrange("p (e j) -> p j e", e=E),
                    in_=d["psum_e"].rearrange("p (j e) -> p j e", e=E),
                )
                nc.scalar.dma_start(
                    out=mhTd[b][H + 1 : KM, :],
                    in_=me_sb.rearrange("p (e j) -> p e j", e=E),
                )

            def s3(b):
                psum_msgT = ppool_msg.tile([H, JB], f32, tag="m", name="psum_msgT")
                nc.tensor.matmul(
                    psum_msgT[:, :], lhsT=WWb[:, :], rhs=mhTd[b][:, :],
                    start=True, stop=True,
                )
                xt = XT[b % NXT]
                nc.vector.tensor_tensor(
                    out=xt[0:H, :],
                    in0=psum_msgT[:, :],
                    in1=hsT_all[:, b * JB : (b + 1) * JB],
                    op=Alu.add,
                )
                psum_outT = ppool_out.tile([H, JB], f32, tag="o", name="psum_outT")
                nc.tensor.matmul(
                    psum_outT[:, :], lhsT=UUb[:, :], rhs=xt[:, :],
                    start=True, stop=True,
                )
                out_sb = spool.tile([H, JB], f32, name="out_sb")
                nc.scalar.copy(out_sb[:, :], psum_outT[:, :])
                nc.scalar.dma_start(out=out[b], in_=out_sb[:, :])

            LAG2, LAG3 = 1, 2
            for t in range(B + LAG3):
                if t < B:
                    s1(t)
                if LAG2 <= t < B + LAG2:
                    s2(t - LAG2)
                if LAG3 <= t:
                    s3(t - LAG3)

    nc.compile()
    return nc


def _get_program():
    if "nc" not in _CACHE:
        _CACHE["nc"] = _build_program()
    return _CACHE["nc"]


def _make_in_maps(h, edge_attr, adj, W_w, W_b, U_w, U_b):
    h = np.ascontiguousarray(np.asarray(h, dtype=np.float32))
    edge_attr = np.asarray(edge_attr, dtype=np.float32)
    adj = np.asarray(adj, dtype=np.int32)
    W_w = np.ascontiguousarray(np.asarray(W_w, dtype=np.float32))
    W_b = np.ascontiguousarray(np.asarray(W_b, dtype=np.float32)).reshape(1, H)
    U_w = np.ascontiguousarray(np.asarray(U_w, dtype=np.float32))
    U_b = np.ascontiguousarray(np.asarray(U_b, dtype=np.float32)).reshape(1, H)

    in_maps = []
    for c in range(NCORES):
        j0 = c * JB
        in_maps.append(
            {
                "edge": np.ascontiguousarray(edge_attr[:, :, j0 : j0 + JB, :]),
                "adjs": np.ascontiguousarray(adj[:, :, j0 : j0 + JB]),
                "h": h,
                "hs": np.ascontiguousarray(h[:, j0 : j0 + JB, :]),
                "Ww": W_w,
                "Wb": W_b,
                "Uw": U_w,
                "Ub": U_b,
            }
        )
    return in_maps


def _install_ntff_hook():
    """The agent image lacks antenv.axon_hooks; synthesize it so trace=True
    can reach the libaxon NTFF profiling entry points."""
    import sys
    import types

    try:
        from antenv.axon_hooks import get_axon_ntff_profile_hook  # noqa: F401

        return
    except ImportError:
        pass
    import antenv

    mod = types.ModuleType("antenv.axon_hooks")
    _h = [None]
    mod.set_axon_ntff_profile_hook = lambda hook: _h.__setitem__(0, hook)
    mod.get_axon_ntff_profile_hook = lambda: _h[0]
    sys.modules["antenv.axon_hooks"] = mod
    antenv.axon_hooks = mod
    try:
        from trn_agent_boot.trn_boot import _ntff_profile_via_ctypes

        mod.set_axon_ntff_profile_hook(
            _ntff_profile_via_ctypes("/opt/axon/libaxon_pjrt.so")
        )
    except Exception:
        pass
    # avoid the bucket upload (no bucket in this container)
    import concourse.bass_utils as bu

    bu.upload_artifacts = lambda tmpdir: str(tmpdir)


def run(h, edge_attr, adj, W_w, W_b, U_w, U_b, trace=False, trace_cores=None):
    """Run the kernel; returns (output, BassKernelResults)."""
    _ensure_path()
    if trace:
        _install_ntff_hook()
    from concourse.bass_utils import run_bass_kernel_spmd

    nc = _get_program()
    in_maps = _make_in_maps(h, edge_attr, adj, W_w, W_b, U_w, U_b)
    kw = {}
    if trace:
        kw = {"trace": True, "trace_cores": trace_cores or [0]}
    res = run_bass_kernel_spmd(nc, in_maps, list(range(NCORES)), **kw)
    outs = [res.results[c]["out"].transpose(0, 2, 1) for c in range(NCORES)]
    full = np.concatenate(outs, axis=1)  # [B, N, H]
    return full, res


def kernel(h, edge_attr, adj, W_w, W_b, U_w, U_b):
    full, _ = run(h, edge_attr, adj, W_w, W_b, U_w, U_b)
    return full
